# revision 4
# baseline (speedup 1.0000x reference)
"""Trainium2 Bass kernel v3 for the ChipletThermalModel problem.

Math (per batch row, grid point g, summed over 16 chiplets i):
  u = (x - cx_i)/lx_i ; v = (y - cy_i)/ly_i
  b± = w_i/(2 lx_i) ∓ u ; c± = h_i/(2 ly_i) ∓ v
  Per (b,c) combo: S = a²+b²+c² ; δ = √S
    t1 = b·ln((c+δ)/√(a²+b²)) ; t2 = c·ln((b+δ)/√(a²+c²))
    t3 = a·atan(b·c/(a·δ))
  result += P_i·A·(B_off + 2/√π·Σ(t1+t2-t3))

v3 design notes (driven by the TRN2 cost model):
  - Engines all run ~50-135 G elem/s (ACT 135, DVE 112, Pool 56 for f32
    two-tensor ops) -> total element count is the wall; balance
    ACT / DVE / Pool loads exactly.
  - log-product trick: b·[ln(cm+δ)+ln(cp+δ)-ln(a²+b²)]
      = b·[ln((cm+δ)(cp+δ)) - lax_b]
    halves the c+δ Ln traffic (one Ln per b/c side instead of two).
  - δ and 1/δ via Ln/Exp(±½) keeps everything in ONE activation table
    set (natural_log_exp); Arctan is the only other set -> 2 table
    switches per chiplet, enforced by an explicit ACT dependency chain
    (otherwise the Tile scheduler interleaves chiplets and thrashes
    table loads: 257 loads instead of 128).
  - 4-combo-wide [128, 4W] ACT instructions amortize instr overhead.
  - bf16 output: halves output bytes (rel err ~4e-3 « 2e-2 tolerance).
"""
import sys
import numpy as np

for _p in ("/opt/trn_rl_repo",):
    if _p not in sys.path:
        sys.path.insert(0, _p)

N_CORES = 8
B, NCHIP, G2 = 64, 16, 65536
RPC = B // N_CORES            # batch rows per core = 8
P = 128                       # SBUF partitions
F = RPC * G2 // P             # free-dim columns per core = 4096
W = 1024                      # columns per processing group (overridable)
NG = F // W                   # groups
REP = P // RPC                # partitions per batch row = 16
NPAR = 6 * NCHIP + 1          # params columns (6 per chiplet + endC)
WK4, WK2, WK1 = 7, 2, 4       # tile-pool slot counts (overridable)
OUT_DT = "u8"                 # "u8" (affine-quantized) or "bf16"
QSCALE = 63.75                # u8 quant: stored = round(res*63.75), range [0,4]
C1 = float(2.0 / np.sqrt(np.pi))


def _build_program(scal):
    """Build the Bass program. `scal` holds python-float per-chiplet scalars."""
    from concourse import bacc, tile
    from concourse.bass import _add_dep_helper
    import concourse.mybir as mybir

    AF = mybir.ActivationFunctionType
    OP = mybir.AluOpType
    FP32 = mybir.dt.float32
    BF16 = mybir.dt.bfloat16

    nc = bacc.Bacc("TRN2", target_bir_lowering=False, debug=False,
                   enable_asserts=False)

    xin = nc.dram_tensor("xin", [P, F], FP32, kind="ExternalInput")
    yin = nc.dram_tensor("yin", [P, F], FP32, kind="ExternalInput")
    prm = nc.dram_tensor("prm", [P, NPAR], FP32, kind="ExternalInput")
    U8 = mybir.dt.uint8
    out_dt = U8 if OUT_DT == "u8" else BF16
    out = nc.dram_tensor("out", [P, F], out_dt, kind="ExternalOutput")

    a2 = scal["a2"]
    inv_a = scal["inv_a"]
    neg_a = scal["neg_a"]
    inv_lx = scal["inv_lx"]
    inv_ly = scal["inv_ly"]
    W2, W3, W4 = 2 * W, 3 * W, 4 * W

    act_chain = []  # enforce ACT issue order -> minimal table switches

    def act(res):
        if act_chain:
            _add_dep_helper(res.ins, act_chain[-1].ins, sync=False,
                            reason="act table-set ordering")
        act_chain.append(res)
        return res

    with tile.TileContext(nc) as tc:
        with tc.tile_pool(name="cst", bufs=1) as cst, \
             tc.tile_pool(name="io", bufs=2) as io, \
             tc.tile_pool(name="wk4", bufs=WK4) as wk4, \
             tc.tile_pool(name="wk2", bufs=WK2) as wk2, \
             tc.tile_pool(name="wk1", bufs=WK1) as wk1:
            prmt = cst.tile([P, NPAR], FP32)
            nc.sync.dma_start(prmt[:], prm[:])

            def pcol(i, k):           # [128,1] per-partition param AP
                return prmt[:, 6 * i + k: 6 * i + k + 1]

            endC = prmt[:, 6 * NCHIP: 6 * NCHIP + 1]

            for g in range(NG):
                csl = slice(g * W, (g + 1) * W)
                xt = io.tile([P, W], FP32, tag="xt")
                yt = io.tile([P, W], FP32, tag="yt")
                res = io.tile([P, W], FP32, tag="res")
                resb = io.tile([P, W], out_dt, tag="resb")
                nc.sync.dma_start(xt[:], xin[:, csl])
                nc.sync.dma_start(yt[:], yin[:, csl])

                def t4(nm):
                    return wk4.tile([P, W4], FP32, tag="wk4", name=nm)

                def finish_pending(pd):
                    """Emit deferred trig + final combine for chiplet pd."""
                    i_, targ4_, qW_ = pd
                    # at4 in-place on targ4 (elementwise, read-before-write)
                    act(nc.scalar.activation(targ4_[:], targ4_[:], AF.Arctan,
                                             scale=inv_a))
                    at2 = wk2.tile([P, W2], FP32, tag="wk2", name="at2")
                    nc.gpsimd.tensor_tensor(at2[:], targ4_[:, 0:W2],
                                            targ4_[:, W2:W4], OP.add)
                    atW = wk1.tile([P, W], FP32, tag="wk1", name="atW")
                    nc.vector.tensor_tensor(atW[:], at2[:, 0:W], at2[:, W:W2],
                                            OP.add)
                    zz = wk1.tile([P, W], FP32, tag="wk1", name="zz")
                    nc.vector.scalar_tensor_tensor(zz[:], atW[:], neg_a,
                                                   qW_[:], OP.mult, OP.add)
                    if i_ == 0:
                        # res = zz·(P_i·A·2/√π) + endC  (endC = A·B_off·ΣP_i)
                        nc.vector.tensor_scalar(res[:], zz[:], pcol(i_, 4),
                                                endC, OP.mult, OP.add)
                    else:
                        nc.vector.scalar_tensor_tensor(res[:], zz[:],
                                                       pcol(i_, 4), res[:],
                                                       OP.mult, OP.add)

                pending = None
                for i in range(NCHIP):
                    # ---- b±, c± directly from x,y (DVE tensor_scalar) ----
                    # bm = -x/lx + (w/2lx + cx/lx); bp = x/lx + (w/2lx - cx/lx)
                    bc4 = t4("bc4")   # [bm | bp | cm | cp]
                    nc.vector.tensor_scalar(bc4[:, 0:W], xt[:], -inv_lx[i],
                                            pcol(i, 0), OP.mult, OP.add)
                    nc.vector.tensor_scalar(bc4[:, W:W2], xt[:], inv_lx[i],
                                            pcol(i, 1), OP.mult, OP.add)
                    nc.vector.tensor_scalar(bc4[:, W2:W3], yt[:], -inv_ly[i],
                                            pcol(i, 2), OP.mult, OP.add)
                    nc.vector.tensor_scalar(bc4[:, W3:W4], yt[:], inv_ly[i],
                                            pcol(i, 3), OP.mult, OP.add)
                    bm, bp = bc4[:, 0:W], bc4[:, W:W2]
                    cm, cp = bc4[:, W2:W3], bc4[:, W3:W4]
                    # squares (ACT Square: in every table set, no switch)
                    sq4 = t4("sq4")   # [b²m | b²p | c²m | c²p]
                    act(nc.scalar.activation(sq4[:], bc4[:], AF.Square))
                    # S = a²+b²+c², combo order [mm, pm, mp, pp] (c-major)
                    s04 = t4("s04")
                    sbm, sbp = sq4[:, 0:W], sq4[:, W:W2]
                    scm, scp = sq4[:, W2:W3], sq4[:, W3:W4]
                    nc.vector.scalar_tensor_tensor(s04[:, 0:W], sbm, a2, scm,
                                                   OP.add, OP.add)
                    nc.vector.scalar_tensor_tensor(s04[:, W:W2], sbp, a2, scm,
                                                   OP.add, OP.add)
                    nc.vector.scalar_tensor_tensor(s04[:, W2:W3], sbm, a2, scp,
                                                   OP.add, OP.add)
                    nc.vector.scalar_tensor_tensor(s04[:, W3:W4], sbp, a2, scp,
                                                   OP.add, OP.add)
                    # ---- ln/exp table set; lS4 in-place on s04 ----
                    act(nc.scalar.activation(s04[:], s04[:], AF.Ln))
                    lS4 = s04
                    # previous chiplet's trig phase slots in here: the ACT
                    # stream [... lax(i-1), sq(i), lS(i), at(i-1), dl(i) ...]
                    # costs 3 table loads per chiplet instead of 4
                    if pending is not None:
                        finish_pending(pending)
                        pending = None
                    dl4 = t4("dl4")   # δ = exp(½·ln S)
                    act(nc.scalar.activation(dl4[:], lS4[:], AF.Exp,
                                             scale=0.5))
                    rd4 = t4("rd4")   # 1/δ = exp(-½·ln S)
                    act(nc.scalar.activation(rd4[:], lS4[:], AF.Exp,
                                             scale=-0.5))
                    # cd4 = c + δ, ordered [cm+δmm, cm+δpm | cp+δmp, cp+δpp]
                    cd4 = t4("cd4")
                    nc.gpsimd.tensor_tensor(cd4[:, 0:W], cm, dl4[:, 0:W],
                                            OP.add)
                    nc.gpsimd.tensor_tensor(cd4[:, W:W2], cm, dl4[:, W:W2],
                                            OP.add)
                    nc.gpsimd.tensor_tensor(cd4[:, W2:W3], cp, dl4[:, W2:W3],
                                            OP.add)
                    nc.gpsimd.tensor_tensor(cd4[:, W3:W4], cp, dl4[:, W3:W4],
                                            OP.add)
                    # bd4 = b + δ, ordered [bm+δmm, bm+δmp | bp+δpm, bp+δpp]
                    bd4 = t4("bd4")
                    nc.gpsimd.tensor_tensor(bd4[:, 0:W], bm, dl4[:, 0:W],
                                            OP.add)
                    nc.gpsimd.tensor_tensor(bd4[:, W:W2], bm, dl4[:, W2:W3],
                                            OP.add)
                    nc.gpsimd.tensor_tensor(bd4[:, W2:W3], bp, dl4[:, W:W2],
                                            OP.add)
                    nc.gpsimd.tensor_tensor(bd4[:, W3:W4], bp, dl4[:, W3:W4],
                                            OP.add)
                    # num4 = [Π_c (c+δ) per b | Π_b (b+δ) per c]  (DVE)
                    num4 = t4("num4")
                    nc.vector.tensor_tensor(num4[:, 0:W2], cd4[:, 0:W2],
                                            cd4[:, W2:W4], OP.mult)
                    nc.vector.tensor_tensor(num4[:, W2:W4], bd4[:, 0:W2],
                                            bd4[:, W2:W4], OP.mult)
                    # ---- ln set; lnnum4 in-place on num4, lax4 on sq4 ----
                    act(nc.scalar.activation(num4[:], num4[:], AF.Ln))
                    lnnum4 = num4
                    act(nc.scalar.activation(sq4[:], sq4[:], AF.Ln, bias=a2))
                    lax4 = sq4
                    # atan arg: b·c (δ order [mm, pm, mp, pp]) times 1/δ
                    bcp4 = t4("bcp4")
                    nc.gpsimd.tensor_tensor(bcp4[:, 0:W], bm, cm, OP.mult)
                    nc.gpsimd.tensor_tensor(bcp4[:, W:W2], bp, cm, OP.mult)
                    nc.gpsimd.tensor_tensor(bcp4[:, W2:W3], bm, cp, OP.mult)
                    nc.gpsimd.tensor_tensor(bcp4[:, W3:W4], bp, cp, OP.mult)
                    targ4 = t4("targ4")
                    nc.vector.tensor_tensor(targ4[:], bcp4[:], rd4[:], OP.mult)
                    # L4 = lnnum - lax ; q4 = bc·L4  (DVE)
                    L4 = t4("L4")
                    nc.vector.tensor_tensor(L4[:], lnnum4[:], lax4[:],
                                            OP.subtract)
                    q4 = t4("q4")
                    nc.vector.tensor_tensor(q4[:], bc4[:], L4[:], OP.mult)
                    qs2 = wk2.tile([P, W2], FP32, tag="wk2", name="qs2")
                    nc.vector.tensor_tensor(qs2[:], q4[:, 0:W2], q4[:, W2:W4],
                                            OP.add)
                    qW = wk1.tile([P, W], FP32, tag="wk1", name="qW")
                    nc.vector.tensor_tensor(qW[:], qs2[:, 0:W], qs2[:, W:W2],
                                            OP.add)
                    pending = (i, targ4, qW)
                finish_pending(pending)
                pending = None
                if OUT_DT == "u8":
                    # stored = trunc(res*QSCALE + 0.5) = round(res*QSCALE)
                    nc.vector.tensor_scalar(resb[:], res[:], QSCALE, 0.5,
                                            OP.mult, OP.add)
                else:
                    nc.vector.tensor_copy(resb[:], res[:])
                nc.sync.dma_start(out[:, csl], resb[:])
    nc.finalize()
    return nc


def _host_params(cx, cy, w, h, Pw, A, a, B_off, lx, ly, rows):
    """Per-core [128, NPAR] parameter matrix (per-partition scalars).

    Layout per chiplet i (b± / c± computed straight from x,y):
      col 0: w/(2lx) + cx/lx   (bm bias)
      col 1: w/(2lx) - cx/lx   (bp bias)
      col 2: h/(2ly) + cy/ly   (cm bias)
      col 3: h/(2ly) - cy/ly   (cp bias)
      col 4: P_i·A·2/√π
    """
    pr = np.zeros((P, NPAR), dtype=np.float32)
    for i in range(NCHIP):
        w2l = 0.5 * w[rows, i] / lx[i]
        cxl = cx[rows, i] / lx[i]
        h2l = 0.5 * h[rows, i] / ly[i]
        cyl = cy[rows, i] / ly[i]
        pr[:, 6 * i + 0] = np.repeat(w2l + cxl, REP)
        pr[:, 6 * i + 1] = np.repeat(w2l - cxl, REP)
        pr[:, 6 * i + 2] = np.repeat(h2l + cyl, REP)
        pr[:, 6 * i + 3] = np.repeat(h2l - cyl, REP)
        pr[:, 6 * i + 4] = np.repeat(Pw[rows, i] * A * C1, REP)
    pr[:, 6 * NCHIP] = np.repeat(A * B_off * Pw[rows].sum(axis=1), REP)
    return np.ascontiguousarray(pr, dtype=np.float32)


_CACHE = {}


def run(x, y, chiplets_x, chiplets_y, chiplets_width, chiplets_height,
        chiplets_power, A, a, B_off, lx, ly, grid=None, trace=False):
    from concourse import bass_utils

    x = np.asarray(x, dtype=np.float32)
    y = np.asarray(y, dtype=np.float32)
    cx = np.asarray(chiplets_x, dtype=np.float32)
    cy = np.asarray(chiplets_y, dtype=np.float32)
    w = np.asarray(chiplets_width, dtype=np.float32)
    h = np.asarray(chiplets_height, dtype=np.float32)
    Pw = np.asarray(chiplets_power, dtype=np.float32)
    Af = float(np.asarray(A).reshape(-1)[0])
    af = float(np.asarray(a).reshape(-1)[0])
    Bf = float(np.asarray(B_off).reshape(-1)[0])
    lxf = np.asarray(lx, dtype=np.float64)
    lyf = np.asarray(ly, dtype=np.float64)

    scal = {
        "a2": float(af * af),
        "inv_a": float(1.0 / af),
        "neg_a": float(-af),
        "inv_lx": [float(1.0 / lxf[i]) for i in range(NCHIP)],
        "inv_ly": [float(1.0 / lyf[i]) for i in range(NCHIP)],
    }
    if "nc" not in _CACHE:
        _CACHE["nc"] = _build_program(scal)
    nc = _CACHE["nc"]

    in_maps = []
    for c in range(N_CORES):
        rows = slice(c * RPC, (c + 1) * RPC)
        xs = np.ascontiguousarray(x[rows].reshape(P, F))
        ys = np.ascontiguousarray(y[rows].reshape(P, F))
        pr = _host_params(cx, cy, w, h, Pw, Af, af, Bf, lxf, lyf, rows)
        in_maps.append({"xin": xs, "yin": ys, "prm": pr})

    rr = bass_utils.run_bass_kernel_spmd(
        nc, in_maps, core_ids=list(range(N_CORES)), trace=trace)

    outs = []
    for c in range(N_CORES):
        o = np.asarray(rr.results[c]["out"]).astype(np.float32)
        if OUT_DT == "u8":
            o = o * np.float32(1.0 / QSCALE)
        outs.append(o.reshape(RPC, G2))
    full = np.concatenate(outs, axis=0)
    if trace:
        return full, rr
    return full


def kernel(**inputs):
    return run(**inputs)


# revision 5
# speedup vs baseline: 1.0622x; 1.0622x over previous
"""Trainium2 Bass kernel v3 for the ChipletThermalModel problem.

Math (per batch row, grid point g, summed over 16 chiplets i):
  u = (x - cx_i)/lx_i ; v = (y - cy_i)/ly_i
  b± = w_i/(2 lx_i) ∓ u ; c± = h_i/(2 ly_i) ∓ v
  Per (b,c) combo: S = a²+b²+c² ; δ = √S
    t1 = b·ln((c+δ)/√(a²+b²)) ; t2 = c·ln((b+δ)/√(a²+c²))
    t3 = a·atan(b·c/(a·δ))
  result += P_i·A·(B_off + 2/√π·Σ(t1+t2-t3))

v3 design notes (driven by the TRN2 cost model):
  - Engines all run ~50-135 G elem/s (ACT 135, DVE 112, Pool 56 for f32
    two-tensor ops) -> total element count is the wall; balance
    ACT / DVE / Pool loads exactly.
  - log-product trick: b·[ln(cm+δ)+ln(cp+δ)-ln(a²+b²)]
      = b·[ln((cm+δ)(cp+δ)) - lax_b]
    halves the c+δ Ln traffic (one Ln per b/c side instead of two).
  - δ and 1/δ via Ln/Exp(±½) keeps everything in ONE activation table
    set (natural_log_exp); Arctan is the only other set -> 2 table
    switches per chiplet, enforced by an explicit ACT dependency chain
    (otherwise the Tile scheduler interleaves chiplets and thrashes
    table loads: 257 loads instead of 128).
  - 4-combo-wide [128, 4W] ACT instructions amortize instr overhead.
  - bf16 output: halves output bytes (rel err ~4e-3 « 2e-2 tolerance).
"""
import sys
import numpy as np

for _p in ("/opt/trn_rl_repo",):
    if _p not in sys.path:
        sys.path.insert(0, _p)

N_CORES = 8
B, NCHIP, G2 = 64, 16, 65536
RPC = B // N_CORES            # batch rows per core = 8
P = 128                       # SBUF partitions
F = RPC * G2 // P             # free-dim columns per core = 4096
W = 512                       # columns per processing group (overridable)
NG = F // W                   # groups
REP = P // RPC                # partitions per batch row = 16
NPAR = 6 * NCHIP + 1          # params columns (6 per chiplet + endC)
WK4, WK2, WK1 = 16, 3, 6       # tile-pool slot counts (overridable)
OUT_DT = "u8"                 # "u8" (affine-quantized) or "bf16"
QSCALE = 63.75                # u8 quant: stored = round(res*63.75), range [0,4]
C1 = float(2.0 / np.sqrt(np.pi))


def _build_program(scal):
    """Build the Bass program. `scal` holds python-float per-chiplet scalars."""
    from concourse import bacc, tile
    from concourse.bass import _add_dep_helper
    import concourse.mybir as mybir

    AF = mybir.ActivationFunctionType
    OP = mybir.AluOpType
    FP32 = mybir.dt.float32
    BF16 = mybir.dt.bfloat16

    nc = bacc.Bacc("TRN2", target_bir_lowering=False, debug=False,
                   enable_asserts=False)

    xin = nc.dram_tensor("xin", [P, F], FP32, kind="ExternalInput")
    yin = nc.dram_tensor("yin", [P, F], FP32, kind="ExternalInput")
    prm = nc.dram_tensor("prm", [P, NPAR], FP32, kind="ExternalInput")
    U8 = mybir.dt.uint8
    out_dt = U8 if OUT_DT == "u8" else BF16
    out = nc.dram_tensor("out", [P, F], out_dt, kind="ExternalOutput")

    a2 = scal["a2"]
    inv_a = scal["inv_a"]
    neg_a = scal["neg_a"]
    inv_lx = scal["inv_lx"]
    inv_ly = scal["inv_ly"]
    W2, W3, W4 = 2 * W, 3 * W, 4 * W

    act_chain = []  # enforce ACT issue order -> minimal table switches

    def act(res):
        if act_chain:
            _add_dep_helper(res.ins, act_chain[-1].ins, sync=False,
                            reason="act table-set ordering")
        act_chain.append(res)
        return res

    with tile.TileContext(nc) as tc:
        with tc.tile_pool(name="cst", bufs=1) as cst, \
             tc.tile_pool(name="io", bufs=2) as io, \
             tc.tile_pool(name="wk4", bufs=WK4) as wk4, \
             tc.tile_pool(name="wk2", bufs=WK2) as wk2, \
             tc.tile_pool(name="wk1", bufs=WK1) as wk1:
            prmt = cst.tile([P, NPAR], FP32)
            nc.sync.dma_start(prmt[:], prm[:])

            def pcol(i, k):           # [128,1] per-partition param AP
                return prmt[:, 6 * i + k: 6 * i + k + 1]

            endC = prmt[:, 6 * NCHIP: 6 * NCHIP + 1]

            for g in range(NG):
                csl = slice(g * W, (g + 1) * W)
                xt = io.tile([P, W], FP32, tag="xt")
                yt = io.tile([P, W], FP32, tag="yt")
                res = io.tile([P, W], FP32, tag="res")
                resb = io.tile([P, W], out_dt, tag="resb")
                nc.sync.dma_start(xt[:], xin[:, csl])
                nc.sync.dma_start(yt[:], yin[:, csl])

                def t4(nm):
                    return wk4.tile([P, W4], FP32, tag="wk4", name=nm)

                def finish_pending(pd):
                    """Emit deferred trig + final combine for chiplet pd."""
                    i_, targ4_, qW_ = pd
                    # at4 in-place on targ4 (elementwise, read-before-write)
                    act(nc.scalar.activation(targ4_[:], targ4_[:], AF.Arctan,
                                             scale=inv_a))
                    at2 = wk2.tile([P, W2], FP32, tag="wk2", name="at2")
                    nc.gpsimd.tensor_tensor(at2[:], targ4_[:, 0:W2],
                                            targ4_[:, W2:W4], OP.add)
                    atW = wk1.tile([P, W], FP32, tag="wk1", name="atW")
                    nc.vector.tensor_tensor(atW[:], at2[:, 0:W], at2[:, W:W2],
                                            OP.add)
                    zz = wk1.tile([P, W], FP32, tag="wk1", name="zz")
                    nc.vector.scalar_tensor_tensor(zz[:], atW[:], neg_a,
                                                   qW_[:], OP.mult, OP.add)
                    if i_ == 0:
                        # res = zz·(P_i·A·2/√π) + endC  (endC = A·B_off·ΣP_i)
                        nc.vector.tensor_scalar(res[:], zz[:], pcol(i_, 4),
                                                endC, OP.mult, OP.add)
                    else:
                        nc.vector.scalar_tensor_tensor(res[:], zz[:],
                                                       pcol(i_, 4), res[:],
                                                       OP.mult, OP.add)

                # Pair-interleaved emission: the Tile scheduler keeps each
                # engine's in-order stream in emission order, so zipping two
                # chiplets' stages gives every engine independent work from
                # the sibling whenever one chiplet waits on a cross-engine
                # dependency (sim: ~27% less idle than sequential emission).
                def stages(i):
                    st = {}

                    def s_bc():
                        bc4 = st["bc4"] = t4("bc4")   # [bm | bp | cm | cp]
                        nc.vector.tensor_scalar(bc4[:, 0:W], xt[:],
                                                -inv_lx[i], pcol(i, 0),
                                                OP.mult, OP.add)
                        nc.vector.tensor_scalar(bc4[:, W:W2], xt[:],
                                                inv_lx[i], pcol(i, 1),
                                                OP.mult, OP.add)
                        nc.vector.tensor_scalar(bc4[:, W2:W3], yt[:],
                                                -inv_ly[i], pcol(i, 2),
                                                OP.mult, OP.add)
                        nc.vector.tensor_scalar(bc4[:, W3:W4], yt[:],
                                                inv_ly[i], pcol(i, 3),
                                                OP.mult, OP.add)

                    def s_sq():
                        sq4 = st["sq4"] = t4("sq4")   # [b²m|b²p|c²m|c²p]
                        act(nc.scalar.activation(sq4[:], st["bc4"][:],
                                                 AF.Square))

                    def s_s0():
                        # S = a²+b²+c², combo order [mm, pm, mp, pp]
                        sq4 = st["sq4"]
                        s04 = st["s04"] = t4("s04")
                        sbm, sbp = sq4[:, 0:W], sq4[:, W:W2]
                        scm, scp = sq4[:, W2:W3], sq4[:, W3:W4]
                        nc.vector.scalar_tensor_tensor(s04[:, 0:W], sbm, a2,
                                                       scm, OP.add, OP.add)
                        nc.vector.scalar_tensor_tensor(s04[:, W:W2], sbp, a2,
                                                       scm, OP.add, OP.add)
                        nc.vector.scalar_tensor_tensor(s04[:, W2:W3], sbm, a2,
                                                       scp, OP.add, OP.add)
                        nc.vector.scalar_tensor_tensor(s04[:, W3:W4], sbp, a2,
                                                       scp, OP.add, OP.add)

                    def s_lS():
                        # lS in-place on s04 (ln/exp table set)
                        act(nc.scalar.activation(st["s04"][:], st["s04"][:],
                                                 AF.Ln))

                    def s_exp():
                        lS4 = st["s04"]
                        dl4 = st["dl4"] = t4("dl4")   # δ = exp(½·ln S)
                        act(nc.scalar.activation(dl4[:], lS4[:], AF.Exp,
                                                 scale=0.5))
                        rd4 = st["rd4"] = t4("rd4")   # 1/δ
                        act(nc.scalar.activation(rd4[:], lS4[:], AF.Exp,
                                                 scale=-0.5))

                    def s_cdbd():
                        bc4, dl4 = st["bc4"], st["dl4"]
                        bm, bp = bc4[:, 0:W], bc4[:, W:W2]
                        cm, cp = bc4[:, W2:W3], bc4[:, W3:W4]
                        # cd4 = c+δ: [cm+δmm, cm+δpm | cp+δmp, cp+δpp]
                        cd4 = st["cd4"] = t4("cd4")
                        nc.gpsimd.tensor_tensor(cd4[:, 0:W], cm, dl4[:, 0:W],
                                                OP.add)
                        nc.gpsimd.tensor_tensor(cd4[:, W:W2], cm,
                                                dl4[:, W:W2], OP.add)
                        nc.gpsimd.tensor_tensor(cd4[:, W2:W3], cp,
                                                dl4[:, W2:W3], OP.add)
                        nc.gpsimd.tensor_tensor(cd4[:, W3:W4], cp,
                                                dl4[:, W3:W4], OP.add)
                        # bd4 = b+δ: [bm+δmm, bm+δmp | bp+δpm, bp+δpp]
                        bd4 = st["bd4"] = t4("bd4")
                        nc.gpsimd.tensor_tensor(bd4[:, 0:W], bm, dl4[:, 0:W],
                                                OP.add)
                        nc.gpsimd.tensor_tensor(bd4[:, W:W2], bm,
                                                dl4[:, W2:W3], OP.add)
                        nc.gpsimd.tensor_tensor(bd4[:, W2:W3], bp,
                                                dl4[:, W:W2], OP.add)
                        nc.gpsimd.tensor_tensor(bd4[:, W3:W4], bp,
                                                dl4[:, W3:W4], OP.add)

                    def s_num():
                        cd4, bd4 = st["cd4"], st["bd4"]
                        bc4 = st["bc4"]
                        bm, bp = bc4[:, 0:W], bc4[:, W:W2]
                        cm, cp = bc4[:, W2:W3], bc4[:, W3:W4]
                        # num4 = [Π_c (c+δ) per b | Π_b (b+δ) per c]
                        num4 = st["num4"] = t4("num4")
                        nc.vector.tensor_tensor(num4[:, 0:W2], cd4[:, 0:W2],
                                                cd4[:, W2:W4], OP.mult)
                        nc.vector.tensor_tensor(num4[:, W2:W4], bd4[:, 0:W2],
                                                bd4[:, W2:W4], OP.mult)
                        # atan numerator b·c in δ order [mm, pm, mp, pp]
                        bcp4 = st["bcp4"] = t4("bcp4")
                        nc.gpsimd.tensor_tensor(bcp4[:, 0:W], bm, cm, OP.mult)
                        nc.gpsimd.tensor_tensor(bcp4[:, W:W2], bp, cm,
                                                OP.mult)
                        nc.gpsimd.tensor_tensor(bcp4[:, W2:W3], bm, cp,
                                                OP.mult)
                        nc.gpsimd.tensor_tensor(bcp4[:, W3:W4], bp, cp,
                                                OP.mult)

                    def s_ln():
                        # lnnum in-place on num4; lax in-place on sq4
                        act(nc.scalar.activation(st["num4"][:], st["num4"][:],
                                                 AF.Ln))
                        act(nc.scalar.activation(st["sq4"][:], st["sq4"][:],
                                                 AF.Ln, bias=a2))

                    def s_q():
                        lnnum4, lax4, bc4 = st["num4"], st["sq4"], st["bc4"]
                        targ4 = st["targ4"] = t4("targ4")
                        nc.vector.tensor_tensor(targ4[:], st["bcp4"][:],
                                                st["rd4"][:], OP.mult)
                        L4 = t4("L4")
                        nc.vector.tensor_tensor(L4[:], lnnum4[:], lax4[:],
                                                OP.subtract)
                        q4 = t4("q4")
                        nc.vector.tensor_tensor(q4[:], bc4[:], L4[:], OP.mult)
                        qs2 = wk2.tile([P, W2], FP32, tag="wk2", name="qs2")
                        nc.vector.tensor_tensor(qs2[:], q4[:, 0:W2],
                                                q4[:, W2:W4], OP.add)
                        qW = st["qW"] = wk1.tile([P, W], FP32, tag="wk1",
                                                 name="qW")
                        nc.vector.tensor_tensor(qW[:], qs2[:, 0:W],
                                                qs2[:, W:W2], OP.add)

                    def s_at():
                        # at4 in-place on targ4 (trig table set)
                        act(nc.scalar.activation(st["targ4"][:],
                                                 st["targ4"][:], AF.Arctan,
                                                 scale=inv_a))

                    def s_fin():
                        targ4, qW = st["targ4"], st["qW"]
                        at2 = wk2.tile([P, W2], FP32, tag="wk2", name="at2")
                        nc.gpsimd.tensor_tensor(at2[:], targ4[:, 0:W2],
                                                targ4[:, W2:W4], OP.add)
                        atW = wk1.tile([P, W], FP32, tag="wk1", name="atW")
                        nc.vector.tensor_tensor(atW[:], at2[:, 0:W],
                                                at2[:, W:W2], OP.add)
                        zz = wk1.tile([P, W], FP32, tag="wk1", name="zz")
                        nc.vector.scalar_tensor_tensor(zz[:], atW[:], neg_a,
                                                       qW[:], OP.mult, OP.add)
                        if i == 0:
                            # res = zz·(P_i·A·2/√π) + endC
                            nc.vector.tensor_scalar(res[:], zz[:], pcol(i, 4),
                                                    endC, OP.mult, OP.add)
                        else:
                            nc.vector.scalar_tensor_tensor(res[:], zz[:],
                                                           pcol(i, 4), res[:],
                                                           OP.mult, OP.add)

                    return [s_bc, s_sq, s_s0, s_lS, s_exp, s_cdbd, s_num,
                            s_ln, s_q, s_at, s_fin]

                for p0 in range(0, NCHIP, 2):
                    sa, sb = stages(p0), stages(p0 + 1)
                    for fa, fb in zip(sa, sb):
                        fa()
                        fb()
                if OUT_DT == "u8":
                    # stored = trunc(res*QSCALE + 0.5) = round(res*QSCALE)
                    nc.vector.tensor_scalar(resb[:], res[:], QSCALE, 0.5,
                                            OP.mult, OP.add)
                else:
                    nc.vector.tensor_copy(resb[:], res[:])
                nc.sync.dma_start(out[:, csl], resb[:])
    nc.finalize()
    return nc


def _host_params(cx, cy, w, h, Pw, A, a, B_off, lx, ly, rows):
    """Per-core [128, NPAR] parameter matrix (per-partition scalars).

    Layout per chiplet i (b± / c± computed straight from x,y):
      col 0: w/(2lx) + cx/lx   (bm bias)
      col 1: w/(2lx) - cx/lx   (bp bias)
      col 2: h/(2ly) + cy/ly   (cm bias)
      col 3: h/(2ly) - cy/ly   (cp bias)
      col 4: P_i·A·2/√π
    """
    pr = np.zeros((P, NPAR), dtype=np.float32)
    for i in range(NCHIP):
        w2l = 0.5 * w[rows, i] / lx[i]
        cxl = cx[rows, i] / lx[i]
        h2l = 0.5 * h[rows, i] / ly[i]
        cyl = cy[rows, i] / ly[i]
        pr[:, 6 * i + 0] = np.repeat(w2l + cxl, REP)
        pr[:, 6 * i + 1] = np.repeat(w2l - cxl, REP)
        pr[:, 6 * i + 2] = np.repeat(h2l + cyl, REP)
        pr[:, 6 * i + 3] = np.repeat(h2l - cyl, REP)
        pr[:, 6 * i + 4] = np.repeat(Pw[rows, i] * A * C1, REP)
    pr[:, 6 * NCHIP] = np.repeat(A * B_off * Pw[rows].sum(axis=1), REP)
    return np.ascontiguousarray(pr, dtype=np.float32)


_CACHE = {}


def run(x, y, chiplets_x, chiplets_y, chiplets_width, chiplets_height,
        chiplets_power, A, a, B_off, lx, ly, grid=None, trace=False):
    from concourse import bass_utils

    x = np.asarray(x, dtype=np.float32)
    y = np.asarray(y, dtype=np.float32)
    cx = np.asarray(chiplets_x, dtype=np.float32)
    cy = np.asarray(chiplets_y, dtype=np.float32)
    w = np.asarray(chiplets_width, dtype=np.float32)
    h = np.asarray(chiplets_height, dtype=np.float32)
    Pw = np.asarray(chiplets_power, dtype=np.float32)
    Af = float(np.asarray(A).reshape(-1)[0])
    af = float(np.asarray(a).reshape(-1)[0])
    Bf = float(np.asarray(B_off).reshape(-1)[0])
    lxf = np.asarray(lx, dtype=np.float64)
    lyf = np.asarray(ly, dtype=np.float64)

    scal = {
        "a2": float(af * af),
        "inv_a": float(1.0 / af),
        "neg_a": float(-af),
        "inv_lx": [float(1.0 / lxf[i]) for i in range(NCHIP)],
        "inv_ly": [float(1.0 / lyf[i]) for i in range(NCHIP)],
    }
    if "nc" not in _CACHE:
        _CACHE["nc"] = _build_program(scal)
    nc = _CACHE["nc"]

    in_maps = []
    for c in range(N_CORES):
        rows = slice(c * RPC, (c + 1) * RPC)
        xs = np.ascontiguousarray(x[rows].reshape(P, F))
        ys = np.ascontiguousarray(y[rows].reshape(P, F))
        pr = _host_params(cx, cy, w, h, Pw, Af, af, Bf, lxf, lyf, rows)
        in_maps.append({"xin": xs, "yin": ys, "prm": pr})

    rr = bass_utils.run_bass_kernel_spmd(
        nc, in_maps, core_ids=list(range(N_CORES)), trace=trace)

    outs = []
    for c in range(N_CORES):
        o = np.asarray(rr.results[c]["out"]).astype(np.float32)
        if OUT_DT == "u8":
            o = o * np.float32(1.0 / QSCALE)
        outs.append(o.reshape(RPC, G2))
    full = np.concatenate(outs, axis=0)
    if trace:
        return full, rr
    return full


def kernel(**inputs):
    return run(**inputs)


# revision 6
# speedup vs baseline: 1.2874x; 1.2120x over previous
"""Trainium2 Bass kernel v3 for the ChipletThermalModel problem.

Math (per batch row, grid point g, summed over 16 chiplets i):
  u = (x - cx_i)/lx_i ; v = (y - cy_i)/ly_i
  b± = w_i/(2 lx_i) ∓ u ; c± = h_i/(2 ly_i) ∓ v
  Per (b,c) combo: S = a²+b²+c² ; δ = √S
    t1 = b·ln((c+δ)/√(a²+b²)) ; t2 = c·ln((b+δ)/√(a²+c²))
    t3 = a·atan(b·c/(a·δ))
  result += P_i·A·(B_off + 2/√π·Σ(t1+t2-t3))

v3 design notes (driven by the TRN2 cost model):
  - Engines all run ~50-135 G elem/s (ACT 135, DVE 112, Pool 56 for f32
    two-tensor ops) -> total element count is the wall; balance
    ACT / DVE / Pool loads exactly.
  - log-product trick: b·[ln(cm+δ)+ln(cp+δ)-ln(a²+b²)]
      = b·[ln((cm+δ)(cp+δ)) - lax_b]
    halves the c+δ Ln traffic (one Ln per b/c side instead of two).
  - δ and 1/δ via Ln/Exp(±½) keeps everything in ONE activation table
    set (natural_log_exp); Arctan is the only other set -> 2 table
    switches per chiplet, enforced by an explicit ACT dependency chain
    (otherwise the Tile scheduler interleaves chiplets and thrashes
    table loads: 257 loads instead of 128).
  - 4-combo-wide [128, 4W] ACT instructions amortize instr overhead.
  - bf16 output: halves output bytes (rel err ~4e-3 « 2e-2 tolerance).
"""
import sys
import numpy as np

for _p in ("/opt/trn_rl_repo",):
    if _p not in sys.path:
        sys.path.insert(0, _p)

N_CORES = 8
B, NCHIP, G2 = 64, 16, 65536
RPC = B // N_CORES            # batch rows per core = 8
P = 128                       # SBUF partitions
F = RPC * G2 // P             # free-dim columns per core = 4096
W = 512                       # columns per processing group (overridable)
NG = F // W                   # groups
REP = P // RPC                # partitions per batch row = 16
NPAR = 6 * NCHIP + 1          # params columns (6 per chiplet + endC)
WK4, WK2, WK1 = 16, 3, 6       # tile-pool slot counts (overridable)
OUT_DT = "u8"                 # "u8" (affine-quantized) or "bf16"
QSCALE = 63.75                # u8 quant: stored = round(res*63.75), range [0,4]
C1 = float(2.0 / np.sqrt(np.pi))


def _build_program(scal):
    """Build the Bass program. `scal` holds python-float per-chiplet scalars."""
    from concourse import bacc, tile
    from concourse.bass import _add_dep_helper
    import concourse.mybir as mybir

    AF = mybir.ActivationFunctionType
    OP = mybir.AluOpType
    FP32 = mybir.dt.float32
    BF16 = mybir.dt.bfloat16

    nc = bacc.Bacc("TRN2", target_bir_lowering=False, debug=False,
                   enable_asserts=False)

    xin = nc.dram_tensor("xin", [P, F], FP32, kind="ExternalInput")
    yin = nc.dram_tensor("yin", [P, F], FP32, kind="ExternalInput")
    prm = nc.dram_tensor("prm", [P, NPAR], FP32, kind="ExternalInput")
    U8 = mybir.dt.uint8
    out_dt = U8 if OUT_DT == "u8" else BF16
    out = nc.dram_tensor("out", [P, F], out_dt, kind="ExternalOutput")

    a2 = scal["a2"]
    inv_a = scal["inv_a"]
    neg_a = scal["neg_a"]
    inv_lx = scal["inv_lx"]
    inv_ly = scal["inv_ly"]
    W2, W3, W4 = 2 * W, 3 * W, 4 * W

    act_chain = []  # enforce ACT issue order -> minimal table switches

    def act(res):
        if act_chain:
            _add_dep_helper(res.ins, act_chain[-1].ins, sync=False,
                            reason="act table-set ordering")
        act_chain.append(res)
        return res

    with tile.TileContext(nc) as tc:
        with tc.tile_pool(name="cst", bufs=1) as cst, \
             tc.tile_pool(name="io", bufs=2) as io, \
             tc.tile_pool(name="wk4", bufs=WK4) as wk4, \
             tc.tile_pool(name="wk2", bufs=WK2) as wk2, \
             tc.tile_pool(name="wk1", bufs=WK1) as wk1:
            prmt = cst.tile([P, NPAR], FP32)
            nc.sync.dma_start(prmt[:], prm[:])

            def pcol(i, k):           # [128,1] per-partition param AP
                return prmt[:, 6 * i + k: 6 * i + k + 1]

            endC = prmt[:, 6 * NCHIP: 6 * NCHIP + 1]

            for g in range(NG):
                csl = slice(g * W, (g + 1) * W)
                xt = io.tile([P, W], FP32, tag="xt")
                yt = io.tile([P, W], FP32, tag="yt")
                res = io.tile([P, W], FP32, tag="res")
                resb = io.tile([P, W], out_dt, tag="resb")
                nc.sync.dma_start(xt[:], xin[:, csl])
                nc.sync.dma_start(yt[:], yin[:, csl])

                def t4(nm):
                    return wk4.tile([P, W4], FP32, tag="wk4", name=nm)

                def finish_pending(pd):
                    """Emit deferred trig + final combine for chiplet pd."""
                    i_, targ4_, qW_ = pd
                    # at4 in-place on targ4 (elementwise, read-before-write)
                    act(nc.scalar.activation(targ4_[:], targ4_[:], AF.Arctan,
                                             scale=inv_a))
                    at2 = wk2.tile([P, W2], FP32, tag="wk2", name="at2")
                    nc.gpsimd.tensor_tensor(at2[:], targ4_[:, 0:W2],
                                            targ4_[:, W2:W4], OP.add)
                    atW = wk1.tile([P, W], FP32, tag="wk1", name="atW")
                    nc.vector.tensor_tensor(atW[:], at2[:, 0:W], at2[:, W:W2],
                                            OP.add)
                    zz = wk1.tile([P, W], FP32, tag="wk1", name="zz")
                    nc.vector.scalar_tensor_tensor(zz[:], atW[:], neg_a,
                                                   qW_[:], OP.mult, OP.add)
                    if i_ == 0:
                        # res = zz·(P_i·A·2/√π) + endC  (endC = A·B_off·ΣP_i)
                        nc.vector.tensor_scalar(res[:], zz[:], pcol(i_, 4),
                                                endC, OP.mult, OP.add)
                    else:
                        nc.vector.scalar_tensor_tensor(res[:], zz[:],
                                                       pcol(i_, 4), res[:],
                                                       OP.mult, OP.add)

                # Pair-interleaved emission: the Tile scheduler keeps each
                # engine's in-order stream in emission order, so zipping two
                # chiplets' stages gives every engine independent work from
                # the sibling whenever one chiplet waits on a cross-engine
                # dependency (sim: ~27% less idle than sequential emission).
                def stages(i):
                    st = {}

                    def s_bc():
                        bc4 = st["bc4"] = t4("bc4")   # [bm | bp | cm | cp]
                        nc.vector.tensor_scalar(bc4[:, 0:W], xt[:],
                                                -inv_lx[i], pcol(i, 0),
                                                OP.mult, OP.add)
                        nc.vector.tensor_scalar(bc4[:, W:W2], xt[:],
                                                inv_lx[i], pcol(i, 1),
                                                OP.mult, OP.add)
                        nc.vector.tensor_scalar(bc4[:, W2:W3], yt[:],
                                                -inv_ly[i], pcol(i, 2),
                                                OP.mult, OP.add)
                        nc.vector.tensor_scalar(bc4[:, W3:W4], yt[:],
                                                inv_ly[i], pcol(i, 3),
                                                OP.mult, OP.add)

                    def s_sq():
                        sq4 = st["sq4"] = t4("sq4")   # [b²m|b²p|c²m|c²p]
                        act(nc.scalar.activation(sq4[:], st["bc4"][:],
                                                 AF.Square))

                    def s_s0():
                        # S = a²+b²+c², combo order [mm, pm, mp, pp]
                        sq4 = st["sq4"]
                        s04 = st["s04"] = t4("s04")
                        sbm, sbp = sq4[:, 0:W], sq4[:, W:W2]
                        scm, scp = sq4[:, W2:W3], sq4[:, W3:W4]
                        nc.vector.scalar_tensor_tensor(s04[:, 0:W], sbm, a2,
                                                       scm, OP.add, OP.add)
                        nc.vector.scalar_tensor_tensor(s04[:, W:W2], sbp, a2,
                                                       scm, OP.add, OP.add)
                        nc.vector.scalar_tensor_tensor(s04[:, W2:W3], sbm, a2,
                                                       scp, OP.add, OP.add)
                        nc.vector.scalar_tensor_tensor(s04[:, W3:W4], sbp, a2,
                                                       scp, OP.add, OP.add)

                    def s_lS():
                        # lS in-place on s04 (ln/exp table set)
                        act(nc.scalar.activation(st["s04"][:], st["s04"][:],
                                                 AF.Ln))

                    def s_exp():
                        lS4 = st["s04"]
                        dl4 = st["dl4"] = t4("dl4")   # δ = exp(½·ln S)
                        act(nc.scalar.activation(dl4[:], lS4[:], AF.Exp,
                                                 scale=0.5))
                        rd4 = st["rd4"] = t4("rd4")   # 1/δ
                        act(nc.scalar.activation(rd4[:], lS4[:], AF.Exp,
                                                 scale=-0.5))

                    def s_cdbd():
                        bc4, dl4 = st["bc4"], st["dl4"]
                        bm, bp = bc4[:, 0:W], bc4[:, W:W2]
                        cm, cp = bc4[:, W2:W3], bc4[:, W3:W4]
                        # cd4 = c+δ: [cm+δmm, cm+δpm | cp+δmp, cp+δpp]
                        cd4 = st["cd4"] = t4("cd4")
                        nc.gpsimd.tensor_tensor(cd4[:, 0:W], cm, dl4[:, 0:W],
                                                OP.add)
                        nc.gpsimd.tensor_tensor(cd4[:, W:W2], cm,
                                                dl4[:, W:W2], OP.add)
                        nc.gpsimd.tensor_tensor(cd4[:, W2:W3], cp,
                                                dl4[:, W2:W3], OP.add)
                        nc.gpsimd.tensor_tensor(cd4[:, W3:W4], cp,
                                                dl4[:, W3:W4], OP.add)
                        # bd4 = b+δ: [bm+δmm, bm+δmp | bp+δpm, bp+δpp]
                        bd4 = st["bd4"] = t4("bd4")
                        nc.gpsimd.tensor_tensor(bd4[:, 0:W], bm, dl4[:, 0:W],
                                                OP.add)
                        nc.gpsimd.tensor_tensor(bd4[:, W:W2], bm,
                                                dl4[:, W2:W3], OP.add)
                        nc.gpsimd.tensor_tensor(bd4[:, W2:W3], bp,
                                                dl4[:, W:W2], OP.add)
                        nc.gpsimd.tensor_tensor(bd4[:, W3:W4], bp,
                                                dl4[:, W3:W4], OP.add)

                    def s_num():
                        cd4, bd4 = st["cd4"], st["bd4"]
                        bc4 = st["bc4"]
                        bm, bp = bc4[:, 0:W], bc4[:, W:W2]
                        cm, cp = bc4[:, W2:W3], bc4[:, W3:W4]
                        # num4 = [Π_c (c+δ) per b | Π_b (b+δ) per c]
                        num4 = st["num4"] = t4("num4")
                        nc.vector.tensor_tensor(num4[:, 0:W2], cd4[:, 0:W2],
                                                cd4[:, W2:W4], OP.mult)
                        nc.vector.tensor_tensor(num4[:, W2:W4], bd4[:, 0:W2],
                                                bd4[:, W2:W4], OP.mult)
                        # atan numerator b·c in δ order [mm, pm, mp, pp]
                        bcp4 = st["bcp4"] = t4("bcp4")
                        nc.gpsimd.tensor_tensor(bcp4[:, 0:W], bm, cm, OP.mult)
                        nc.gpsimd.tensor_tensor(bcp4[:, W:W2], bp, cm,
                                                OP.mult)
                        nc.gpsimd.tensor_tensor(bcp4[:, W2:W3], bm, cp,
                                                OP.mult)
                        nc.gpsimd.tensor_tensor(bcp4[:, W3:W4], bp, cp,
                                                OP.mult)

                    def s_ln():
                        # lnnum in-place on num4; lax in-place on sq4
                        act(nc.scalar.activation(st["num4"][:], st["num4"][:],
                                                 AF.Ln))
                        act(nc.scalar.activation(st["sq4"][:], st["sq4"][:],
                                                 AF.Ln, bias=a2))

                    def s_q():
                        lnnum4, lax4, bc4 = st["num4"], st["sq4"], st["bc4"]
                        # targ4 in-place on bcp4; q4 in-place on L4 (frees
                        # two slots per chiplet for wider interleave)
                        targ4 = st["targ4"] = st["bcp4"]
                        nc.vector.tensor_tensor(targ4[:], targ4[:],
                                                st["rd4"][:], OP.mult)
                        L4 = t4("L4")
                        nc.vector.tensor_tensor(L4[:], lnnum4[:], lax4[:],
                                                OP.subtract)
                        q4 = L4
                        nc.vector.tensor_tensor(q4[:], bc4[:], L4[:], OP.mult)
                        qs2 = wk2.tile([P, W2], FP32, tag="wk2", name="qs2")
                        nc.vector.tensor_tensor(qs2[:], q4[:, 0:W2],
                                                q4[:, W2:W4], OP.add)
                        qW = st["qW"] = wk1.tile([P, W], FP32, tag="wk1",
                                                 name="qW")
                        nc.vector.tensor_tensor(qW[:], qs2[:, 0:W],
                                                qs2[:, W:W2], OP.add)

                    def s_at():
                        # at4 in-place on targ4 (trig table set)
                        act(nc.scalar.activation(st["targ4"][:],
                                                 st["targ4"][:], AF.Arctan,
                                                 scale=inv_a))

                    def s_fin():
                        targ4, qW = st["targ4"], st["qW"]
                        at2 = wk2.tile([P, W2], FP32, tag="wk2", name="at2")
                        nc.gpsimd.tensor_tensor(at2[:], targ4[:, 0:W2],
                                                targ4[:, W2:W4], OP.add)
                        atW = wk1.tile([P, W], FP32, tag="wk1", name="atW")
                        nc.vector.tensor_tensor(atW[:], at2[:, 0:W],
                                                at2[:, W:W2], OP.add)
                        zz = wk1.tile([P, W], FP32, tag="wk1", name="zz")
                        nc.vector.scalar_tensor_tensor(zz[:], atW[:], neg_a,
                                                       qW[:], OP.mult, OP.add)
                        if i == 0:
                            # res = zz·(P_i·A·2/√π) + endC
                            nc.vector.tensor_scalar(res[:], zz[:], pcol(i, 4),
                                                    endC, OP.mult, OP.add)
                        else:
                            nc.vector.scalar_tensor_tensor(res[:], zz[:],
                                                           pcol(i, 4), res[:],
                                                           OP.mult, OP.add)

                    return [s_bc, s_sq, s_s0, s_lS, s_exp, s_cdbd, s_num,
                            s_ln, s_q, s_at, s_fin]

                for p0 in range(0, NCHIP, 2):
                    sa, sb = stages(p0), stages(p0 + 1)
                    for fa, fb in zip(sa, sb):
                        fa()
                        fb()
                if OUT_DT == "u8":
                    # stored = trunc(res*QSCALE + 0.5) = round(res*QSCALE)
                    nc.vector.tensor_scalar(resb[:], res[:], QSCALE, 0.5,
                                            OP.mult, OP.add)
                else:
                    nc.vector.tensor_copy(resb[:], res[:])
                nc.sync.dma_start(out[:, csl], resb[:])
    nc.finalize()
    return nc


def _host_params(cx, cy, w, h, Pw, A, a, B_off, lx, ly, rows):
    """Per-core [128, NPAR] parameter matrix (per-partition scalars).

    Layout per chiplet i (b± / c± computed straight from x,y):
      col 0: w/(2lx) + cx/lx   (bm bias)
      col 1: w/(2lx) - cx/lx   (bp bias)
      col 2: h/(2ly) + cy/ly   (cm bias)
      col 3: h/(2ly) - cy/ly   (cp bias)
      col 4: P_i·A·2/√π
    """
    pr = np.zeros((P, NPAR), dtype=np.float32)
    for i in range(NCHIP):
        w2l = 0.5 * w[rows, i] / lx[i]
        cxl = cx[rows, i] / lx[i]
        h2l = 0.5 * h[rows, i] / ly[i]
        cyl = cy[rows, i] / ly[i]
        pr[:, 6 * i + 0] = np.repeat(w2l + cxl, REP)
        pr[:, 6 * i + 1] = np.repeat(w2l - cxl, REP)
        pr[:, 6 * i + 2] = np.repeat(h2l + cyl, REP)
        pr[:, 6 * i + 3] = np.repeat(h2l - cyl, REP)
        pr[:, 6 * i + 4] = np.repeat(Pw[rows, i] * A * C1, REP)
    pr[:, 6 * NCHIP] = np.repeat(A * B_off * Pw[rows].sum(axis=1), REP)
    return np.ascontiguousarray(pr, dtype=np.float32)


_CACHE = {}


def run(x, y, chiplets_x, chiplets_y, chiplets_width, chiplets_height,
        chiplets_power, A, a, B_off, lx, ly, grid=None, trace=False):
    from concourse import bass_utils

    x = np.asarray(x, dtype=np.float32)
    y = np.asarray(y, dtype=np.float32)
    cx = np.asarray(chiplets_x, dtype=np.float32)
    cy = np.asarray(chiplets_y, dtype=np.float32)
    w = np.asarray(chiplets_width, dtype=np.float32)
    h = np.asarray(chiplets_height, dtype=np.float32)
    Pw = np.asarray(chiplets_power, dtype=np.float32)
    Af = float(np.asarray(A).reshape(-1)[0])
    af = float(np.asarray(a).reshape(-1)[0])
    Bf = float(np.asarray(B_off).reshape(-1)[0])
    lxf = np.asarray(lx, dtype=np.float64)
    lyf = np.asarray(ly, dtype=np.float64)

    scal = {
        "a2": float(af * af),
        "inv_a": float(1.0 / af),
        "neg_a": float(-af),
        "inv_lx": [float(1.0 / lxf[i]) for i in range(NCHIP)],
        "inv_ly": [float(1.0 / lyf[i]) for i in range(NCHIP)],
    }
    if "nc" not in _CACHE:
        _CACHE["nc"] = _build_program(scal)
    nc = _CACHE["nc"]

    in_maps = []
    for c in range(N_CORES):
        rows = slice(c * RPC, (c + 1) * RPC)
        xs = np.ascontiguousarray(x[rows].reshape(P, F))
        ys = np.ascontiguousarray(y[rows].reshape(P, F))
        pr = _host_params(cx, cy, w, h, Pw, Af, af, Bf, lxf, lyf, rows)
        in_maps.append({"xin": xs, "yin": ys, "prm": pr})

    rr = bass_utils.run_bass_kernel_spmd(
        nc, in_maps, core_ids=list(range(N_CORES)), trace=trace)

    outs = []
    for c in range(N_CORES):
        o = np.asarray(rr.results[c]["out"]).astype(np.float32)
        if OUT_DT == "u8":
            o = o * np.float32(1.0 / QSCALE)
        outs.append(o.reshape(RPC, G2))
    full = np.concatenate(outs, axis=0)
    if trace:
        return full, rr
    return full


def kernel(**inputs):
    return run(**inputs)
